# revision 26
# baseline (speedup 1.0000x reference)
"""Trainium2 Bass kernel for AdvancedSimilarityComputation (retrieval_knn).

Sharding: candidates (N=16384) split across 8 NeuronCores (2048 each).
Each core computes the full query projection (replicated) plus its candidate
shard's projections and the [B, N_local] similarity block; softmax over the
full N axis needs one tiny AllReduce of per-(query, head) exp-sums.

Layout: activations live TRANSPOSED — [d_model on partitions (8 blocks of
128), rows on free] — so every linear layer is matmul(out=Y^T, lhsT=W
(natural layout), rhs=X^T) with no per-layer transposes; only the raw
inputs are transposed once on the PE.  kp^T/kh^T are staged to DRAM and
streamed back per 512-candidate chunk (SBUF pressure).

The fusion MLP runs on the PE: L1 as one block-diagonal [6->128] matmul per
b-row pair (dual-row packing), relu-evicted to bf16 alternating ACT/DVE;
L2 uses the h1 tile as the matmul STATIONARY with block-diag [w2;w2] as the
moving operand so outputs land DENSE as [n-partitions, b-free].  The device
output is therefore [N_local, B]; the host transposes.
"""

import numpy as np
from contextlib import ExitStack

import concourse.bass as bass
import concourse.tile as tile
from concourse import bacc, mybir
from concourse.bass_utils import run_bass_kernel_spmd
from concourse.masks import make_identity

F32 = mybir.dt.float32
BF16 = mybir.dt.bfloat16
AF = mybir.ActivationFunctionType
ALU = mybir.AluOpType

B = 1024          # queries
D = 1024          # d_model
N = 16384         # candidates (global)
NCORES = 8
NL = N // NCORES  # candidates per core
H = 8
HD = D // H
P = 128
DB = D // P       # 8 d-model blocks
SCALE = 1.0 / float(np.sqrt(HD))
EPS = 1e-5
FH = 64           # fusion hidden

WNAMES = [
    "temperature",
    "q_w1", "q_b1", "q_g", "q_be", "q_w2", "q_b2",
    "k_w1", "k_b1", "k_g", "k_be", "k_w2", "k_b2",
    "wq", "bq", "wk", "bk",
    "f_w1", "f_b1", "f_w2", "f_b2",
]


def _bcast_ap(src_ap, nparts):
    """Partition-broadcast a [1, ...] AP to nparts partitions (step 0)."""
    return bass.AP(
        tensor=src_ap.tensor,
        offset=src_ap.offset,
        ap=[[0, nparts]] + [list(p) for p in src_ap.ap[1:]],
    )


def build_program():
    nc = bacc.Bacc("TRN2", target_bir_lowering=False, debug=False,
                   num_devices=NCORES)

    dram = {}
    dram["query_features"] = nc.dram_tensor(
        "query_features", [B, D], F32, kind="ExternalInput").ap()
    dram["candidate_features"] = nc.dram_tensor(
        "candidate_features", [NL, D], F32, kind="ExternalInput").ap()
    shapes = {
        "temperature": [1],
        "q_w1": [D, D], "q_b1": [D], "q_g": [D], "q_be": [D],
        "q_w2": [D, D], "q_b2": [D],
        "k_w1": [D, D], "k_b1": [D], "k_g": [D], "k_be": [D],
        "k_w2": [D, D], "k_b2": [D],
        "wq": [D, D], "bq": [D], "wk": [D, D], "bk": [D],
        "f_w1": [3, FH], "f_b1": [FH], "f_w2": [FH, 1], "f_b2": [1],
    }
    for k in WNAMES:
        dram[k] = nc.dram_tensor(k, shapes[k], F32, kind="ExternalInput").ap()
    # transposed on-device layout [NL, B]; the host transposes + concats
    out_dram = nc.dram_tensor("out", [NL, B], F32, kind="ExternalOutput").ap()

    with tile.TileContext(nc) as tc:
        with nc.allow_low_precision(reason="bf16 pipeline validated offline"):
            _build(nc, tc, dram, out_dram)
    nc.compile()
    return nc


def _build(nc, tc, dram, out_dram):
    ctx = ExitStack()
    pool_const = ctx.enter_context(tc.tile_pool(name="const", bufs=1))
    pool_big = ctx.enter_context(tc.tile_pool(name="big", bufs=1))
    ps_mm = ctx.enter_context(tc.tile_pool(name="ps_mm", bufs=3, space="PSUM"))
    dram_pool = ctx.enter_context(tc.tile_pool(name="dramp", bufs=1,
                                               space="DRAM"))

    def bcast_rows(row_ap, dst, tag):
        d = dram_pool.tile([1, row_ap.shape[-1]], row_ap.dtype, tag=tag,
                           name=f"bd_{tag}")
        nc.sync.dma_start(out=d, in_=row_ap)
        nc.gpsimd.dma_start(out=dst, in_=_bcast_ap(d[:], P))

    # ---- constants ----
    ident = pool_const.tile([P, P], F32)
    make_identity(nc, ident)
    ones_bf = pool_const.tile([P, 1], BF16)
    nc.vector.memset(ones_bf, 1.0)
    eps_t = pool_const.tile([1, 1], F32)
    nc.vector.memset(eps_t, EPS)

    def load_colvec(vap):
        t = pool_const.tile([P, vap.shape[0] // P], F32,
                            name=f"cv_{vap.tensor.name}")
        nc.sync.dma_start(out=t, in_=vap.rearrange("(blk p) -> p blk", p=P))
        return t

    b1q = load_colvec(dram["q_b1"]); gq = load_colvec(dram["q_g"])
    beq = load_colvec(dram["q_be"]); b2q = load_colvec(dram["q_b2"])
    b1k = load_colvec(dram["k_b1"]); gk = load_colvec(dram["k_g"])
    bek = load_colvec(dram["k_be"]); b2k = load_colvec(dram["k_b2"])
    bqc = load_colvec(dram["bq"]); bkc = load_colvec(dram["bk"])

    # fusion weights: block-diag dual-row L1 [6,128] and L2 moving [w2;w2]
    fw1bf = pool_const.tile([6, P], F32)
    nc.vector.memset(fw1bf, 0.0)
    nc.sync.dma_start(out=fw1bf[0:3, 0:FH], in_=dram["f_w1"])
    nc.sync.dma_start(out=fw1bf[3:6, FH:P], in_=dram["f_w1"])
    fw1blk = pool_const.tile([6, P], BF16)
    nc.vector.tensor_copy(out=fw1blk, in_=fw1bf)
    fw2f = pool_const.tile([P, 2], F32)
    nc.vector.memset(fw2f, 0.0)
    nc.sync.dma_start(out=fw2f[0:FH, 0:1], in_=dram["f_w2"])
    nc.sync.dma_start(out=fw2f[FH:P, 1:2], in_=dram["f_w2"])
    fw2 = pool_const.tile([P, 2], BF16)
    nc.vector.tensor_copy(out=fw2, in_=fw2f)
    fb1 = pool_const.tile([P, 1], F32)
    nc.sync.dma_start(out=fb1[0:FH, :],
                      in_=dram["f_b1"].rearrange("(p o) -> p o", o=1))
    nc.sync.dma_start(out=fb1[FH:P, :],
                      in_=dram["f_b1"].rearrange("(p o) -> p o", o=1))
    fb2 = pool_const.tile([P, 1], F32)
    nc.gpsimd.dma_start(out=fb2, in_=_bcast_ap(
        dram["f_b2"].rearrange("(o p) -> o p", o=1), P))

    tmp_t = pool_const.tile([1, 1], F32)
    nc.sync.dma_start(out=tmp_t,
                      in_=dram["temperature"].rearrange("(o p) -> o p", o=1))
    et_row = pool_const.tile([1, 1], F32)
    nc.scalar.activation(et_row, tmp_t, AF.Exp)
    et_b = pool_const.tile([P, 1], F32)
    bcast_rows(et_row, et_b, "et")

    # ---- persistent SBUF (q side) + DRAM staging (k side) ----
    qpT = pool_big.tile([P, DB, B], BF16)
    qhT = pool_big.tile([P, DB, B], BF16)
    kp_dram = dram_pool.tile([P, DB, NL], BF16)
    kh_dram = dram_pool.tile([P, DB, NL], BF16)

    # norm-derived constants (filled in phase A)
    invq_et = pool_const.tile([P, B // P], F32)
    qsq_col = pool_const.tile([P, B // P], F32)
    ksq_b = pool_const.tile([P, NL], BF16)
    ivk_b = pool_const.tile([P, NL], BF16)

    # =====================================================================
    # PHASE A: projections (phase-scoped pools)
    # =====================================================================
    with ExitStack() as actx:
        pool_w = actx.enter_context(tc.tile_pool(name="wpool", bufs=2))
        pool_stage = actx.enter_context(tc.tile_pool(name="stage", bufs=2))
        pool_x = actx.enter_context(tc.tile_pool(name="xpool", bufs=1))
        pool_t1 = actx.enter_context(tc.tile_pool(name="t1", bufs=1))
        pool_work = actx.enter_context(tc.tile_pool(name="workA", bufs=2))
        pool_small = actx.enter_context(tc.tile_pool(name="smallA", bufs=1))
        pool_bc = actx.enter_context(tc.tile_pool(name="bcA", bufs=2))
        pool_kch = actx.enter_context(tc.tile_pool(name="kch", bufs=2))
        ps_tr = actx.enter_context(tc.tile_pool(name="ps_tr", bufs=2,
                                                space="PSUM"))
        ps_stat = actx.enter_context(tc.tile_pool(name="ps_stat", bufs=1,
                                                  space="PSUM"))

        qsq_row = pool_small.tile([1, B], F32, tag="qsq_row")
        ksq_row = pool_small.tile([1, NL], F32, tag="ksq_row")

        def load_weight(wap, slot):
            wt = pool_w.tile([P, DB, D], BF16, tag="wcur",
                             name=f"w_{wap.tensor.name}_{slot}")
            for kb in range(DB):
                st = pool_stage.tile([P, D], F32, tag="stg")
                nc.sync.dma_start(out=st, in_=wap[kb * P:(kb + 1) * P, :])
                nc.any.tensor_copy(out=wt[:, kb, :], in_=st)
            return wt

        def transpose_input(xap, r0, R, name):
            xT = pool_x.tile([P, DB, R], BF16, tag="xT", name=name)
            for rb in range(R // P):
                st = pool_stage.tile([P, D], F32, tag="stg")
                nc.sync.dma_start(out=st,
                                  in_=xap[r0 + rb * P:r0 + (rb + 1) * P, :])
                for db in range(DB):
                    pt = ps_tr.tile([P, P], F32, tag="pt")
                    nc.tensor.transpose(pt, st[:, db * P:(db + 1) * P], ident)
                    nc.any.tensor_copy(out=xT[:, db, rb * P:(rb + 1) * P],
                                       in_=pt)
            return xT

        def projection(xT, R, cb, w1, w2, b1t, gt, bet, b2t, outT, oc0,
                       sq_row):
            """Project R rows; write outT[:, :, oc0:oc0+R], sq_row[cb:]."""
            t2T = pool_t1.tile([P, DB, R], BF16, tag="t2T",
                               name=f"t2T_{cb}_{outT.name}")
            for rb in range(R // 512):
                cols = slice(rb * 512, (rb + 1) * 512)
                ps_mu = ps_stat.tile([1, 512], F32, tag="ps_mu")
                ps_sq = ps_stat.tile([1, 512], F32, tag="ps_sq")
                t1f = pool_t1.tile([P, DB, 512], BF16, tag="t1f")
                for mb in range(DB):
                    ps = ps_mm.tile([P, 512], F32, tag="ps")
                    for kb in range(DB):
                        nc.tensor.matmul(ps, w1[:, kb, mb * P:(mb + 1) * P],
                                         xT[:, kb, cols],
                                         start=(kb == 0), stop=(kb == DB - 1))
                    nc.scalar.activation(t1f[:, mb, :], ps, AF.Identity,
                                         bias=b1t[:, mb:mb + 1])
                    sq = pool_work.tile([P, 512], BF16, tag="sq")
                    nc.vector.tensor_mul(sq, t1f[:, mb, :], t1f[:, mb, :])
                    nc.tensor.matmul(ps_mu, ones_bf, t1f[:, mb, :],
                                     start=(mb == 0), stop=(mb == DB - 1),
                                     skip_group_check=True)
                    nc.tensor.matmul(ps_sq, ones_bf, sq,
                                     start=(mb == 0), stop=(mb == DB - 1),
                                     skip_group_check=True)
                mu = pool_small.tile([1, 512], F32, tag="mu")
                nc.scalar.mul(mu, ps_mu, 1.0 / D)
                msq = pool_small.tile([1, 512], F32, tag="msq")
                nc.scalar.mul(msq, ps_sq, 1.0 / D)
                var = pool_small.tile([1, 512], F32, tag="var")
                nc.vector.tensor_mul(var, mu, mu)
                nc.vector.tensor_tensor(out=var, in0=msq, in1=var,
                                        op=ALU.subtract)
                sd = pool_small.tile([1, 512], F32, tag="sd")
                nc.scalar.activation(sd, var, AF.Sqrt, bias=eps_t)
                rstd = pool_small.tile([1, 512], F32, tag="rstd")
                nc.vector.reciprocal(rstd, sd)
                mu_bf = pool_small.tile([1, 512], BF16, tag="mu_bf")
                nc.vector.tensor_copy(out=mu_bf, in_=mu)
                rstd_bf = pool_small.tile([1, 512], BF16, tag="rstd_bf")
                nc.vector.tensor_copy(out=rstd_bf, in_=rstd)
                mu_b = pool_bc.tile([P, 512], BF16, tag="mu_b")
                bcast_rows(mu_bf, mu_b, "mu_d")
                rstd_b = pool_bc.tile([P, 512], BF16, tag="rstd_b")
                bcast_rows(rstd_bf, rstd_b, "rstd_d")
                for mb in range(DB):
                    u = pool_work.tile([P, 512], BF16, tag="u")
                    nc.vector.tensor_tensor(out=u, in0=t1f[:, mb, :],
                                            in1=mu_b, op=ALU.subtract)
                    nc.vector.tensor_mul(u, u, rstd_b)
                    nc.scalar.activation(t2T[:, mb, cols], u, AF.Gelu,
                                         bias=bet[:, mb:mb + 1],
                                         scale=gt[:, mb:mb + 1])
            for rb in range(R // 512):
                cols = slice(rb * 512, (rb + 1) * 512)
                wcols = slice(oc0 + rb * 512, oc0 + (rb + 1) * 512)
                scols = slice(cb + rb * 512, cb + (rb + 1) * 512)
                ps_ss = ps_stat.tile([1, 512], F32, tag="ps_mu", name=f"ps_ss_{cb}_{rb}_{outT.name}")
                for mb in range(DB):
                    ps = ps_mm.tile([P, 512], F32, tag="ps")
                    for kb in range(DB):
                        nc.tensor.matmul(ps, w2[:, kb, mb * P:(mb + 1) * P],
                                         t2T[:, kb, cols],
                                         start=(kb == 0), stop=(kb == DB - 1))
                    nc.scalar.activation(outT[:, mb, wcols], ps, AF.Identity,
                                         bias=b2t[:, mb:mb + 1])
                    sq = pool_work.tile([P, 512], BF16, tag="sq")
                    nc.vector.tensor_mul(sq, outT[:, mb, wcols],
                                         outT[:, mb, wcols])
                    nc.tensor.matmul(ps_ss, ones_bf, sq,
                                     start=(mb == 0), stop=(mb == DB - 1),
                                     skip_group_check=True)
                nc.vector.tensor_copy(out=sq_row[0:1, scols], in_=ps_ss)

        def head_proj(xT, R, w, bt_col, outT):
            for rb in range(R // 512):
                cols = slice(rb * 512, (rb + 1) * 512)
                for mb in range(DB):
                    ps = ps_mm.tile([P, 512], F32, tag="ps")
                    for kb in range(DB):
                        nc.tensor.matmul(ps, w[:, kb, mb * P:(mb + 1) * P],
                                         xT[:, kb, cols],
                                         start=(kb == 0), stop=(kb == DB - 1))
                    nc.scalar.activation(outT[:, mb, cols], ps, AF.Identity,
                                         bias=bt_col[:, mb:mb + 1])

        # -- query side (stays in SBUF), 512-row chunks
        w1q = load_weight(dram["q_w1"], 0)
        w2q = load_weight(dram["q_w2"], 1)
        for rch in range(B // 512):
            xqT = transpose_input(dram["query_features"], rch * 512, 512,
                                  f"xqT{rch}")
            projection(xqT, 512, rch * 512, w1q, w2q, b1q, gq, beq, b2q,
                       qpT, rch * 512, qsq_row)

        # -- candidate side, 512-row chunks staged to DRAM
        w1k = load_weight(dram["k_w1"], 2)
        w2k = load_weight(dram["k_w2"], 3)
        for rch in range(NL // 512):
            xcT = transpose_input(dram["candidate_features"], rch * 512, 512,
                                  f"xcT{rch}")
            kch = pool_kch.tile([P, DB, 512], BF16, tag="kch",
                                name=f"kp_ch{rch}")
            projection(xcT, 512, rch * 512, w1k, w2k, b1k, gk, bek, b2k,
                       kch, 0, ksq_row)
            nc.sync.dma_start(out=kp_dram[:, :, rch * 512:(rch + 1) * 512],
                              in_=kch)

        wqw = load_weight(dram["wq"], 4)
        head_proj(qpT, B, wqw, bqc, qhT)
        wkw = load_weight(dram["wk"], 5)
        for rch in range(NL // 512):
            kch = pool_kch.tile([P, DB, 512], BF16, tag="kch",
                                name=f"kp_rd{rch}")
            nc.sync.dma_start(out=kch,
                              in_=kp_dram[:, :, rch * 512:(rch + 1) * 512])
            khch = pool_kch.tile([P, DB, 512], BF16, tag="khch",
                                 name=f"kh_ch{rch}")
            head_proj(kch, 512, wkw, bkc, khch)
            nc.sync.dma_start(out=kh_dram[:, :, rch * 512:(rch + 1) * 512],
                              in_=khch)

        # -- norms / scale vectors (rows derived in place)
        for bt in range(B // P):
            pt2 = ps_tr.tile([P, 1], F32, tag="pt", name=f"pt2_{bt}")
            nc.tensor.transpose(pt2, qsq_row[0:1, bt * P:(bt + 1) * P],
                                ident[0:1, 0:1])
            nc.any.tensor_copy(out=qsq_col[:, bt:bt + 1], in_=pt2)
        ksq_bf = pool_small.tile([1, NL], BF16, tag="ksq_bf")
        nc.vector.tensor_copy(out=ksq_bf, in_=ksq_row)
        bcast_rows(ksq_bf, ksq_b, "ksq_d")
        # overwrite the sq rows with 1/sqrt
        nc.scalar.activation(qsq_row, qsq_row, AF.Sqrt)
        nc.vector.reciprocal(qsq_row, qsq_row)
        nc.scalar.activation(ksq_row, ksq_row, AF.Sqrt)
        nc.vector.reciprocal(ksq_row, ksq_row)
        for bt in range(B // P):
            pt1 = ps_tr.tile([P, 1], F32, tag="pt", name=f"pt1_{bt}")
            nc.tensor.transpose(pt1, qsq_row[0:1, bt * P:(bt + 1) * P],
                                ident[0:1, 0:1])
            nc.scalar.mul(invq_et[:, bt:bt + 1], pt1, et_b[:, 0:1])
        ivk_bf = pool_small.tile([1, NL], BF16, tag="ksq_bf",
                                 name="ivk_bf")
        nc.vector.tensor_copy(out=ivk_bf, in_=ksq_row)
        bcast_rows(ivk_bf, ivk_b, "ivk_d")

    # =====================================================================
    # PHASE B: similarity passes + fusion MLP (nch-outer, k streamed)
    # =====================================================================
    n_bt = B // P
    n_nch = NL // 512
    with ExitStack() as bctx:
        pool_ks = bctx.enter_context(tc.tile_pool(name="ks", bufs=2))
        pool_work = bctx.enter_context(tc.tile_pool(name="workB", bufs=2))
        pool_small = bctx.enter_context(tc.tile_pool(name="smallB", bufs=1))
        pool_sim = bctx.enter_context(tc.tile_pool(name="sim", bufs=2))
        pool_stack = bctx.enter_context(tc.tile_pool(name="stack", bufs=1))
        pool_h1 = bctx.enter_context(tc.tile_pool(name="h1", bufs=1))
        pool_eh = bctx.enter_context(tc.tile_pool(name="eh", bufs=1))
        pool_outs = bctx.enter_context(tc.tile_pool(name="outs", bufs=2))
        ps_h1 = bctx.enter_context(tc.tile_pool(name="ps_h1", bufs=2,
                                                space="PSUM"))
        ps_out = bctx.enter_context(tc.tile_pool(name="ps_out", bufs=2,
                                                 space="PSUM"))

        # ---- pass 1: softmax denominators (nch-outer, khT streamed)
        rs_all = pool_const.tile([P, n_bt * H, n_nch], F32)
        for nch in range(n_nch):
            cols = slice(nch * 512, (nch + 1) * 512)
            khs = pool_ks.tile([P, DB, 512], BF16, tag="khs",
                               name=f"khs_p1_{nch}")
            nc.sync.dma_start(out=khs, in_=kh_dram[:, :, cols])
            for bt in range(n_bt):
                bsl = slice(bt * P, (bt + 1) * P)
                for h in range(H):
                    ps = ps_mm.tile([P, 512], F32, tag="ps")
                    nc.tensor.matmul(ps, qhT[:, h, bsl], khs[:, h, :],
                                     start=True, stop=True)
                    junk = pool_work.tile([P, 512], BF16, tag="junk")
                    nc.scalar.activation(
                        junk, ps, AF.Exp, scale=SCALE,
                        accum_out=rs_all[:, bt * H + h, nch:nch + 1])
        rssum = pool_const.tile([P, n_bt * H], F32)
        nc.vector.tensor_reduce(out=rssum, in_=rs_all,
                                axis=mybir.AxisListType.X, op=ALU.add)

        # ---- collective: AllReduce the denominators
        cc_in = dram_pool.tile([P, n_bt * H], F32)
        cc_out = dram_pool.tile([P, n_bt * H], F32)
        nc.sync.dma_start(out=cc_in, in_=rssum)
        nc.gpsimd.collective_compute(
            "AllReduce", ALU.add,
            replica_groups=[list(range(NCORES))],
            ins=[cc_in.opt()],
            outs=[cc_out.opt()],
        )
        denom = pool_const.tile([P, n_bt * H], F32)
        nc.sync.dma_start(out=denom, in_=cc_out)
        # bias for pass2 exp: -(ln denom) - ln 8 (folds the mean over heads)
        lnd = pool_const.tile([P, n_bt * H], F32)
        nc.scalar.activation(lnd, denom, AF.Ln)
        nc.vector.tensor_scalar(out=lnd, in0=lnd, scalar1=-1.0,
                                scalar2=-float(np.log(H)), op0=ALU.mult,
                                op1=ALU.add)

        # ---- pass 2 + fusion MLP
        for nch in range(n_nch):
            cols = slice(nch * 512, (nch + 1) * 512)
            kps = pool_ks.tile([P, DB, 512], BF16, tag="kps",
                               name=f"kps_{nch}")
            nc.sync.dma_start(out=kps, in_=kp_dram[:, :, cols])
            khs = pool_ks.tile([P, DB, 512], BF16, tag="khs",
                               name=f"khs_p2_{nch}")
            nc.sync.dma_start(out=khs, in_=kh_dram[:, :, cols])
            for bt in range(n_bt):
                bsl = slice(bt * P, (bt + 1) * P)
                cos_t = pool_sim.tile([P, 512], BF16, tag="cos")
                euc_t = pool_sim.tile([P, 512], BF16, tag="euc")
                lrn_t = pool_sim.tile([P, 512], BF16, tag="lrn")
                # dot product
                psd = ps_mm.tile([P, 512], F32, tag="ps", name=f"psd_{bt}_{nch}")
                for kb in range(DB):
                    nc.tensor.matmul(psd, qpT[:, kb, bsl], kps[:, kb, :],
                                     start=(kb == 0), stop=(kb == DB - 1))
                # cosine: dot * (invq*et)[b] * invk[n]
                nc.vector.scalar_tensor_tensor(
                    out=cos_t, in0=psd, scalar=invq_et[:, bt:bt + 1],
                    in1=ivk_b[:, cols], op0=ALU.mult, op1=ALU.mult)
                # euclidean: 1/(1+sqrt(max(qsq+ksq-2dot, 0)))
                t = pool_work.tile([P, 512], F32, tag="eu1")
                nc.vector.scalar_tensor_tensor(
                    out=t, in0=psd, scalar=-2.0, in1=ksq_b[:, cols],
                    op0=ALU.mult, op1=ALU.add)
                nc.vector.tensor_scalar(out=t, in0=t,
                                        scalar1=qsq_col[:, bt:bt + 1],
                                        scalar2=0.0, op0=ALU.add, op1=ALU.max)
                s = pool_work.tile([P, 512], BF16, tag="eu2")
                nc.scalar.activation(s, t, AF.Sqrt)
                nc.vector.tensor_scalar_add(s, s, 1.0)
                nc.vector.reciprocal(euc_t, s)
                # learned: sum_h exp(score*scale - ln(denom*8))
                eh = pool_eh.tile([P, 512, H], BF16, tag="eh")
                for h in range(H):
                    pss = ps_mm.tile([P, 512], F32, tag="ps", name=f"pss_{bt}_{nch}_{h}")
                    nc.tensor.matmul(pss, qhT[:, h, bsl], khs[:, h, :],
                                     start=True, stop=True)
                    nc.scalar.activation(
                        eh[:, :, h], pss, AF.Exp, scale=SCALE,
                        bias=lnd[:, bt * H + h: bt * H + h + 1])
                nc.vector.tensor_reduce(out=lrn_t, in_=eh,
                                        axis=mybir.AxisListType.X, op=ALU.add)

                # ---- fusion MLP (see module docstring)
                pf_all = ps_out.tile([P, 4, P], F32, tag="pf",
                                     name=f"pf_{bt}_{nch}")
                for quar in range(4):    # 16 pairs per quarter
                    st6 = pool_stack.tile([6, 16, 512], BF16, tag="st6",
                                          name=f"st6_{bt}_{nch}_{quar}")
                    row0 = quar * 32
                    for ci, simt in enumerate((cos_t, euc_t, lrn_t)):
                        nc.sync.dma_start(
                            out=st6[ci:ci + 1, :, :],
                            in_=simt[row0:row0 + 32:2, :])
                        nc.sync.dma_start(
                            out=st6[ci + 3:ci + 4, :, :],
                            in_=simt[row0 + 1:row0 + 32:2, :])
                    h1s = []
                    for q in range(16):
                        ph = ps_h1.tile([P, 512], F32, tag="ph",
                                        name=f"ph_{bt}_{nch}_{quar}_{q}")
                        nc.tensor.matmul(ph, fw1blk, st6[:, q, :],
                                         start=True, stop=True)
                        h1 = pool_h1.tile([P, 512], BF16, tag=f"h1_{q}",
                                          name=f"h1_{bt}_{nch}_{quar}_{q}")
                        if q % 2 == 0:
                            nc.scalar.activation(h1, ph, AF.Relu, bias=fb1)
                        else:
                            nc.vector.tensor_scalar(
                                out=h1, in0=ph, scalar1=fb1, scalar2=0.0,
                                op0=ALU.add, op1=ALU.max)
                        h1s.append(h1)
                    for nblk in range(4):
                        bcols = slice(nblk * P, (nblk + 1) * P)
                        for i, h1 in enumerate(h1s):
                            pcol = quar * 32 + 2 * i
                            nc.tensor.matmul(pf_all[:, nblk, pcol:pcol + 2],
                                             h1[:, bcols], fw2,
                                             start=True, stop=True,
                                             skip_group_check=True)
                for nblk in range(4):
                    ot = pool_outs.tile([P, P], F32, tag="ot")
                    nc.scalar.activation(ot, pf_all[:, nblk, :], AF.Sigmoid,
                                         bias=fb2)
                    nc.sync.dma_start(
                        out=out_dram[nch * 512 + nblk * P:
                                     nch * 512 + (nblk + 1) * P, bsl],
                        in_=ot)
    ctx.close()


_CACHED = None


def _get_program():
    global _CACHED
    if _CACHED is None:
        _CACHED = build_program()
    return _CACHED


def _make_in_maps(inputs):
    in_maps = []
    for c in range(NCORES):
        m = {"query_features": np.ascontiguousarray(
                 inputs["query_features"], dtype=np.float32),
             "candidate_features": np.ascontiguousarray(
                 np.asarray(inputs["candidate_features"],
                            dtype=np.float32)[c * NL:(c + 1) * NL])}
        for k in WNAMES:
            v = np.asarray(inputs[k], dtype=np.float32)
            if k == "temperature":
                v = v.reshape([1])
            m[k] = np.ascontiguousarray(v)
        in_maps.append(m)
    return in_maps


def kernel(**inputs):
    nc = _get_program()
    in_maps = _make_in_maps(inputs)
    res = run_bass_kernel_spmd(nc, in_maps, core_ids=list(range(NCORES)))
    return np.concatenate(
        [np.ascontiguousarray(res.results[c]["out"].T) for c in range(NCORES)],
        axis=1)


def run_profiled(inputs):
    """Like kernel() but returns (output, exec_time_ns, trace_path)."""
    import os
    os.environ["BASS_PERFETTO_PROFILE_ALL_CORES"] = "1"
    nc = _get_program()
    in_maps = _make_in_maps(inputs)
    res = run_bass_kernel_spmd(nc, in_maps, core_ids=list(range(NCORES)),
                               trace=True, trace_cores=list(range(NCORES)))
    out = np.concatenate(
        [np.ascontiguousarray(res.results[c]["out"].T) for c in range(NCORES)],
        axis=1)
    tp = res.instructions_and_trace[1] if res.instructions_and_trace else None
    return out, res.exec_time_ns, tp


# revision 28
# speedup vs baseline: 1.3781x; 1.3781x over previous
"""Trainium2 Bass kernel for AdvancedSimilarityComputation (retrieval_knn).

Sharding: candidates (N=16384) split across 8 NeuronCores (2048 each).
Each core computes the full query projection (replicated) plus its candidate
shard's projections and the [B, N_local] similarity block; softmax over the
full N axis needs one tiny AllReduce of per-(query, head) exp-sums.

Layout: activations live TRANSPOSED — [d_model on partitions (8 blocks of
128), rows on free] — so every linear layer is matmul(out=Y^T, lhsT=W
(natural layout), rhs=X^T) with no per-layer transposes; only the raw
inputs are transposed once on the PE.  kp^T/kh^T are staged to DRAM and
streamed back per 512-candidate chunk (SBUF pressure).

The fusion MLP runs on the PE: L1 as one block-diagonal [6->128] matmul per
b-row pair (dual-row packing), relu-evicted to bf16 alternating ACT/DVE;
L2 uses the h1 tile as the matmul STATIONARY with block-diag [w2;w2] as the
moving operand so outputs land DENSE as [n-partitions, b-free].  The device
output is therefore [N_local, B]; the host transposes.
"""

import numpy as np
from contextlib import ExitStack

import concourse.bass as bass
import concourse.tile as tile
from concourse import bacc, mybir
from concourse.bass_utils import run_bass_kernel_spmd
from concourse.masks import make_identity

F32 = mybir.dt.float32
BF16 = mybir.dt.bfloat16
AF = mybir.ActivationFunctionType
ALU = mybir.AluOpType

B = 1024          # queries
D = 1024          # d_model
N = 16384         # candidates (global)
NCORES = 8
NL = N // NCORES  # candidates per core
H = 8
HD = D // H
P = 128
DB = D // P       # 8 d-model blocks
SCALE = 1.0 / float(np.sqrt(HD))
EPS = 1e-5
FH = 64           # fusion hidden

WNAMES = [
    "temperature",
    "q_w1", "q_b1", "q_g", "q_be", "q_w2", "q_b2",
    "k_w1", "k_b1", "k_g", "k_be", "k_w2", "k_b2",
    "wq", "bq", "wk", "bk",
    "f_w1", "f_b1", "f_w2", "f_b2",
]


def _bcast_ap(src_ap, nparts):
    """Partition-broadcast a [1, ...] AP to nparts partitions (step 0)."""
    return bass.AP(
        tensor=src_ap.tensor,
        offset=src_ap.offset,
        ap=[[0, nparts]] + [list(p) for p in src_ap.ap[1:]],
    )


def build_program():
    nc = bacc.Bacc("TRN2", target_bir_lowering=False, debug=False,
                   num_devices=NCORES)

    dram = {}
    dram["query_features"] = nc.dram_tensor(
        "query_features", [B, D], F32, kind="ExternalInput").ap()
    dram["candidate_features"] = nc.dram_tensor(
        "candidate_features", [NL, D], F32, kind="ExternalInput").ap()
    shapes = {
        "temperature": [1],
        "q_w1": [D, D], "q_b1": [D], "q_g": [D], "q_be": [D],
        "q_w2": [D, D], "q_b2": [D],
        "k_w1": [D, D], "k_b1": [D], "k_g": [D], "k_be": [D],
        "k_w2": [D, D], "k_b2": [D],
        "wq": [D, D], "bq": [D], "wk": [D, D], "bk": [D],
        "f_w1": [3, FH], "f_b1": [FH], "f_w2": [FH, 1], "f_b2": [1],
    }
    for k in WNAMES:
        dram[k] = nc.dram_tensor(k, shapes[k], F32, kind="ExternalInput").ap()
    # transposed on-device layout [NL, B]; the host transposes + concats
    out_dram = nc.dram_tensor("out", [NL, B], F32, kind="ExternalOutput").ap()

    with tile.TileContext(nc) as tc:
        with nc.allow_low_precision(reason="bf16 pipeline validated offline"):
            _build(nc, tc, dram, out_dram)
    nc.compile()
    return nc


def _build(nc, tc, dram, out_dram):
    ctx = ExitStack()
    pool_const = ctx.enter_context(tc.tile_pool(name="const", bufs=1))
    pool_big = ctx.enter_context(tc.tile_pool(name="big", bufs=1))
    ps_mm = ctx.enter_context(tc.tile_pool(name="ps_mm", bufs=4, space="PSUM"))
    dram_pool = ctx.enter_context(tc.tile_pool(name="dramp", bufs=1,
                                               space="DRAM"))

    def bcast_rows(row_ap, dst, tag):
        d = dram_pool.tile([1, row_ap.shape[-1]], row_ap.dtype, tag=tag,
                           name=f"bd_{tag}")
        nc.sync.dma_start(out=d, in_=row_ap)
        nc.gpsimd.dma_start(out=dst, in_=_bcast_ap(d[:], P))

    # ---- constants ----
    ident = pool_const.tile([P, P], F32)
    make_identity(nc, ident)
    ones_bf = pool_const.tile([P, 1], BF16)
    nc.vector.memset(ones_bf, 1.0)
    eps_t = pool_const.tile([1, 1], F32)
    nc.vector.memset(eps_t, EPS)

    def load_colvec(vap):
        t = pool_const.tile([P, vap.shape[0] // P], F32,
                            name=f"cv_{vap.tensor.name}")
        nc.sync.dma_start(out=t, in_=vap.rearrange("(blk p) -> p blk", p=P))
        return t

    b1q = load_colvec(dram["q_b1"]); gq = load_colvec(dram["q_g"])
    beq = load_colvec(dram["q_be"]); b2q = load_colvec(dram["q_b2"])
    b1k = load_colvec(dram["k_b1"]); gk = load_colvec(dram["k_g"])
    bek = load_colvec(dram["k_be"]); b2k = load_colvec(dram["k_b2"])
    bqc = load_colvec(dram["bq"]); bkc = load_colvec(dram["bk"])

    # fusion weights: block-diag dual-row L1 [6,128] and L2 moving [w2;w2]
    fw1bf = pool_const.tile([6, P], F32)
    nc.vector.memset(fw1bf, 0.0)
    nc.sync.dma_start(out=fw1bf[0:3, 0:FH], in_=dram["f_w1"])
    nc.sync.dma_start(out=fw1bf[3:6, FH:P], in_=dram["f_w1"])
    fw1blk = pool_const.tile([6, P], BF16)
    nc.vector.tensor_copy(out=fw1blk, in_=fw1bf)
    fw2f = pool_const.tile([P, 2], F32)
    nc.vector.memset(fw2f, 0.0)
    nc.sync.dma_start(out=fw2f[0:FH, 0:1], in_=dram["f_w2"])
    nc.sync.dma_start(out=fw2f[FH:P, 1:2], in_=dram["f_w2"])
    fw2 = pool_const.tile([P, 2], BF16)
    nc.vector.tensor_copy(out=fw2, in_=fw2f)
    fb1 = pool_const.tile([P, 1], F32)
    nc.sync.dma_start(out=fb1[0:FH, :],
                      in_=dram["f_b1"].rearrange("(p o) -> p o", o=1))
    nc.sync.dma_start(out=fb1[FH:P, :],
                      in_=dram["f_b1"].rearrange("(p o) -> p o", o=1))
    fb2 = pool_const.tile([P, 1], F32)
    nc.gpsimd.dma_start(out=fb2, in_=_bcast_ap(
        dram["f_b2"].rearrange("(o p) -> o p", o=1), P))

    tmp_t = pool_const.tile([1, 1], F32)
    nc.sync.dma_start(out=tmp_t,
                      in_=dram["temperature"].rearrange("(o p) -> o p", o=1))
    et_row = pool_const.tile([1, 1], F32)
    nc.scalar.activation(et_row, tmp_t, AF.Exp)
    et_b = pool_const.tile([P, 1], F32)
    bcast_rows(et_row, et_b, "et")

    # ---- persistent SBUF (q side) + DRAM staging (k side) ----
    qpT = pool_big.tile([P, DB, B], BF16)
    qhT = pool_big.tile([P, DB, B], BF16)
    kp_dram = dram_pool.tile([P, DB, NL], BF16)
    kh_dram = dram_pool.tile([P, DB, NL], BF16)

    # norm-derived constants (filled in phase A)
    invq_et = pool_const.tile([P, B // P], F32)
    qsq_col = pool_const.tile([P, B // P], F32)
    ksq_b = pool_const.tile([P, NL], BF16)
    ivk_b = pool_const.tile([P, NL], BF16)

    # =====================================================================
    # PHASE A: projections (phase-scoped pools)
    # =====================================================================
    with ExitStack() as actx:
        pool_w = actx.enter_context(tc.tile_pool(name="wpool", bufs=2))
        pool_stage = actx.enter_context(tc.tile_pool(name="stage", bufs=2))
        pool_x = actx.enter_context(tc.tile_pool(name="xpool", bufs=1))
        pool_t1 = actx.enter_context(tc.tile_pool(name="t1", bufs=1))
        pool_work = actx.enter_context(tc.tile_pool(name="workA", bufs=2))
        pool_small = actx.enter_context(tc.tile_pool(name="smallA", bufs=1))
        pool_bc = actx.enter_context(tc.tile_pool(name="bcA", bufs=2))
        pool_kch = actx.enter_context(tc.tile_pool(name="kch", bufs=2))
        ps_tr = actx.enter_context(tc.tile_pool(name="ps_tr", bufs=2,
                                                space="PSUM"))
        ps_stat = actx.enter_context(tc.tile_pool(name="ps_stat", bufs=1,
                                                  space="PSUM"))

        qsq_row = pool_small.tile([1, B], F32, tag="qsq_row")
        ksq_row = pool_small.tile([1, NL], F32, tag="ksq_row")

        def load_weight(wap, slot):
            wt = pool_w.tile([P, DB, D], BF16, tag="wcur",
                             name=f"w_{wap.tensor.name}_{slot}")
            for kb in range(DB):
                st = pool_stage.tile([P, D], F32, tag="stg")
                nc.sync.dma_start(out=st, in_=wap[kb * P:(kb + 1) * P, :])
                nc.any.tensor_copy(out=wt[:, kb, :], in_=st)
            return wt

        def transpose_input(xap, r0, R, name):
            xT = pool_x.tile([P, DB, R], BF16, tag="xT", name=name)
            for rb in range(R // P):
                st = pool_stage.tile([P, D], F32, tag="stg")
                nc.sync.dma_start(out=st,
                                  in_=xap[r0 + rb * P:r0 + (rb + 1) * P, :])
                for db in range(DB):
                    pt = ps_tr.tile([P, P], F32, tag="pt")
                    nc.tensor.transpose(pt, st[:, db * P:(db + 1) * P], ident)
                    nc.any.tensor_copy(out=xT[:, db, rb * P:(rb + 1) * P],
                                       in_=pt)
            return xT

        def projection(xT, R, cb, w1, w2, b1t, gt, bet, b2t, outT, oc0,
                       sq_row):
            """Project R rows; write outT[:, :, oc0:oc0+R], sq_row[cb:]."""
            t2T = pool_t1.tile([P, DB, R], BF16, tag="t2T",
                               name=f"t2T_{cb}_{outT.name}")
            for rb in range(R // 512):
                cols = slice(rb * 512, (rb + 1) * 512)
                ps_mu = ps_stat.tile([1, 512], F32, tag="ps_mu")
                ps_sq = ps_stat.tile([1, 512], F32, tag="ps_sq")
                t1f = pool_t1.tile([P, DB, 512], BF16, tag="t1f")
                for mb in range(DB):
                    ps = ps_mm.tile([P, 512], F32, tag="ps")
                    for kb in range(DB):
                        nc.tensor.matmul(ps, w1[:, kb, mb * P:(mb + 1) * P],
                                         xT[:, kb, cols],
                                         start=(kb == 0), stop=(kb == DB - 1))
                    nc.scalar.activation(t1f[:, mb, :], ps, AF.Identity,
                                         bias=b1t[:, mb:mb + 1])
                    sq = pool_work.tile([P, 512], BF16, tag="sq")
                    nc.vector.tensor_mul(sq, t1f[:, mb, :], t1f[:, mb, :])
                    nc.tensor.matmul(ps_mu, ones_bf, t1f[:, mb, :],
                                     start=(mb == 0), stop=(mb == DB - 1),
                                     skip_group_check=True)
                    nc.tensor.matmul(ps_sq, ones_bf, sq,
                                     start=(mb == 0), stop=(mb == DB - 1),
                                     skip_group_check=True)
                mu = pool_small.tile([1, 512], F32, tag="mu")
                nc.scalar.mul(mu, ps_mu, 1.0 / D)
                msq = pool_small.tile([1, 512], F32, tag="msq")
                nc.scalar.mul(msq, ps_sq, 1.0 / D)
                var = pool_small.tile([1, 512], F32, tag="var")
                nc.vector.tensor_mul(var, mu, mu)
                nc.vector.tensor_tensor(out=var, in0=msq, in1=var,
                                        op=ALU.subtract)
                sd = pool_small.tile([1, 512], F32, tag="sd")
                nc.scalar.activation(sd, var, AF.Sqrt, bias=eps_t)
                rstd = pool_small.tile([1, 512], F32, tag="rstd")
                nc.vector.reciprocal(rstd, sd)
                mu_bf = pool_small.tile([1, 512], BF16, tag="mu_bf")
                nc.vector.tensor_copy(out=mu_bf, in_=mu)
                rstd_bf = pool_small.tile([1, 512], BF16, tag="rstd_bf")
                nc.vector.tensor_copy(out=rstd_bf, in_=rstd)
                mu_b = pool_bc.tile([P, 512], BF16, tag="mu_b")
                bcast_rows(mu_bf, mu_b, "mu_d")
                rstd_b = pool_bc.tile([P, 512], BF16, tag="rstd_b")
                bcast_rows(rstd_bf, rstd_b, "rstd_d")
                for mb in range(DB):
                    u = pool_work.tile([P, 512], BF16, tag="u")
                    nc.vector.tensor_tensor(out=u, in0=t1f[:, mb, :],
                                            in1=mu_b, op=ALU.subtract)
                    nc.vector.tensor_mul(u, u, rstd_b)
                    nc.scalar.activation(t2T[:, mb, cols], u, AF.Gelu,
                                         bias=bet[:, mb:mb + 1],
                                         scale=gt[:, mb:mb + 1])
            for rb in range(R // 512):
                cols = slice(rb * 512, (rb + 1) * 512)
                wcols = slice(oc0 + rb * 512, oc0 + (rb + 1) * 512)
                scols = slice(cb + rb * 512, cb + (rb + 1) * 512)
                ps_ss = ps_stat.tile([1, 512], F32, tag="ps_mu", name=f"ps_ss_{cb}_{rb}_{outT.name}")
                for mb in range(DB):
                    ps = ps_mm.tile([P, 512], F32, tag="ps")
                    for kb in range(DB):
                        nc.tensor.matmul(ps, w2[:, kb, mb * P:(mb + 1) * P],
                                         t2T[:, kb, cols],
                                         start=(kb == 0), stop=(kb == DB - 1))
                    nc.scalar.activation(outT[:, mb, wcols], ps, AF.Identity,
                                         bias=b2t[:, mb:mb + 1])
                    sq = pool_work.tile([P, 512], BF16, tag="sq")
                    nc.vector.tensor_mul(sq, outT[:, mb, wcols],
                                         outT[:, mb, wcols])
                    nc.tensor.matmul(ps_ss, ones_bf, sq,
                                     start=(mb == 0), stop=(mb == DB - 1),
                                     skip_group_check=True)
                nc.vector.tensor_copy(out=sq_row[0:1, scols], in_=ps_ss)

        def head_proj(xT, R, w, bt_col, outT):
            for rb in range(R // 512):
                cols = slice(rb * 512, (rb + 1) * 512)
                for mb in range(DB):
                    ps = ps_mm.tile([P, 512], F32, tag="ps")
                    for kb in range(DB):
                        nc.tensor.matmul(ps, w[:, kb, mb * P:(mb + 1) * P],
                                         xT[:, kb, cols],
                                         start=(kb == 0), stop=(kb == DB - 1))
                    nc.scalar.activation(outT[:, mb, cols], ps, AF.Identity,
                                         bias=bt_col[:, mb:mb + 1])

        # -- query side (stays in SBUF), 512-row chunks
        w1q = load_weight(dram["q_w1"], 0)
        w2q = load_weight(dram["q_w2"], 1)
        for rch in range(B // 512):
            xqT = transpose_input(dram["query_features"], rch * 512, 512,
                                  f"xqT{rch}")
            projection(xqT, 512, rch * 512, w1q, w2q, b1q, gq, beq, b2q,
                       qpT, rch * 512, qsq_row)

        # -- candidate side, 512-row chunks staged to DRAM
        w1k = load_weight(dram["k_w1"], 2)
        w2k = load_weight(dram["k_w2"], 3)
        for rch in range(NL // 512):
            xcT = transpose_input(dram["candidate_features"], rch * 512, 512,
                                  f"xcT{rch}")
            kch = pool_kch.tile([P, DB, 512], BF16, tag="kch",
                                name=f"kp_ch{rch}")
            projection(xcT, 512, rch * 512, w1k, w2k, b1k, gk, bek, b2k,
                       kch, 0, ksq_row)
            nc.sync.dma_start(out=kp_dram[:, :, rch * 512:(rch + 1) * 512],
                              in_=kch)

        wqw = load_weight(dram["wq"], 4)
        head_proj(qpT, B, wqw, bqc, qhT)
        wkw = load_weight(dram["wk"], 5)
        for rch in range(NL // 512):
            kch = pool_kch.tile([P, DB, 512], BF16, tag="kch",
                                name=f"kp_rd{rch}")
            nc.sync.dma_start(out=kch,
                              in_=kp_dram[:, :, rch * 512:(rch + 1) * 512])
            khch = pool_kch.tile([P, DB, 512], BF16, tag="khch",
                                 name=f"kh_ch{rch}")
            head_proj(kch, 512, wkw, bkc, khch)
            nc.sync.dma_start(out=kh_dram[:, :, rch * 512:(rch + 1) * 512],
                              in_=khch)

        # -- norms / scale vectors (rows derived in place)
        for bt in range(B // P):
            pt2 = ps_tr.tile([P, 1], F32, tag="pt", name=f"pt2_{bt}")
            nc.tensor.transpose(pt2, qsq_row[0:1, bt * P:(bt + 1) * P],
                                ident[0:1, 0:1])
            nc.any.tensor_copy(out=qsq_col[:, bt:bt + 1], in_=pt2)
        ksq_bf = pool_small.tile([1, NL], BF16, tag="ksq_bf")
        nc.vector.tensor_copy(out=ksq_bf, in_=ksq_row)
        bcast_rows(ksq_bf, ksq_b, "ksq_d")
        # overwrite the sq rows with 1/sqrt
        nc.scalar.activation(qsq_row, qsq_row, AF.Sqrt)
        nc.vector.reciprocal(qsq_row, qsq_row)
        nc.scalar.activation(ksq_row, ksq_row, AF.Sqrt)
        nc.vector.reciprocal(ksq_row, ksq_row)
        for bt in range(B // P):
            pt1 = ps_tr.tile([P, 1], F32, tag="pt", name=f"pt1_{bt}")
            nc.tensor.transpose(pt1, qsq_row[0:1, bt * P:(bt + 1) * P],
                                ident[0:1, 0:1])
            nc.scalar.mul(invq_et[:, bt:bt + 1], pt1, et_b[:, 0:1])
        ivk_bf = pool_small.tile([1, NL], BF16, tag="ksq_bf",
                                 name="ivk_bf")
        nc.vector.tensor_copy(out=ivk_bf, in_=ksq_row)
        bcast_rows(ivk_bf, ivk_b, "ivk_d")

    # =====================================================================
    # PHASE B: similarity passes + fusion MLP (nch-outer, k streamed)
    # =====================================================================
    n_bt = B // P
    n_nch = NL // 512
    with ExitStack() as bctx:
        pool_ks = bctx.enter_context(tc.tile_pool(name="ks", bufs=2))
        pool_work = bctx.enter_context(tc.tile_pool(name="workB", bufs=2))
        pool_small = bctx.enter_context(tc.tile_pool(name="smallB", bufs=1))
        pool_sim = bctx.enter_context(tc.tile_pool(name="sim", bufs=3))
        pool_stack = bctx.enter_context(tc.tile_pool(name="stack", bufs=2))
        pool_h1 = bctx.enter_context(tc.tile_pool(name="h1", bufs=2))
        pool_eh = bctx.enter_context(tc.tile_pool(name="eh", bufs=2))
        pool_outs = bctx.enter_context(tc.tile_pool(name="outs", bufs=2))
        ps_h1 = bctx.enter_context(tc.tile_pool(name="ps_h1", bufs=2,
                                                space="PSUM"))
        ps_out = bctx.enter_context(tc.tile_pool(name="ps_out", bufs=2,
                                                 space="PSUM"))

        # ---- pass 1: softmax denominators (nch-outer, khT streamed)
        rs_all = pool_const.tile([P, n_bt * H, n_nch], F32)
        for nch in range(n_nch):
            cols = slice(nch * 512, (nch + 1) * 512)
            khs = pool_ks.tile([P, DB, 512], BF16, tag="khs",
                               name=f"khs_p1_{nch}")
            nc.sync.dma_start(out=khs, in_=kh_dram[:, :, cols])
            for bt in range(n_bt):
                bsl = slice(bt * P, (bt + 1) * P)
                for h in range(H):
                    ps = ps_mm.tile([P, 512], F32, tag="ps")
                    nc.tensor.matmul(ps, qhT[:, h, bsl], khs[:, h, :],
                                     start=True, stop=True)
                    junk = pool_work.tile([P, 512], BF16, tag="junk")
                    nc.scalar.activation(
                        junk, ps, AF.Exp, scale=SCALE,
                        accum_out=rs_all[:, bt * H + h, nch:nch + 1])
        rssum = pool_const.tile([P, n_bt * H], F32)
        nc.vector.tensor_reduce(out=rssum, in_=rs_all,
                                axis=mybir.AxisListType.X, op=ALU.add)

        # ---- collective: AllReduce the denominators
        cc_in = dram_pool.tile([P, n_bt * H], F32)
        cc_out = dram_pool.tile([P, n_bt * H], F32)
        nc.sync.dma_start(out=cc_in, in_=rssum)
        nc.gpsimd.collective_compute(
            "AllReduce", ALU.add,
            replica_groups=[list(range(NCORES))],
            ins=[cc_in.opt()],
            outs=[cc_out.opt()],
        )
        denom = pool_const.tile([P, n_bt * H], F32)
        nc.sync.dma_start(out=denom, in_=cc_out)
        # bias for pass2 exp: -(ln denom) - ln 8 (folds the mean over heads)
        lnd = pool_const.tile([P, n_bt * H], F32)
        nc.scalar.activation(lnd, denom, AF.Ln)
        nc.vector.tensor_scalar(out=lnd, in0=lnd, scalar1=-1.0,
                                scalar2=-float(np.log(H)), op0=ALU.mult,
                                op1=ALU.add)

        # ---- pass 2 + fusion MLP
        for nch in range(n_nch):
            cols = slice(nch * 512, (nch + 1) * 512)
            kps = pool_ks.tile([P, DB, 512], BF16, tag="kps",
                               name=f"kps_{nch}")
            nc.sync.dma_start(out=kps, in_=kp_dram[:, :, cols])
            khs = pool_ks.tile([P, DB, 512], BF16, tag="khs",
                               name=f"khs_p2_{nch}")
            nc.sync.dma_start(out=khs, in_=kh_dram[:, :, cols])
            for bt in range(n_bt):
                bsl = slice(bt * P, (bt + 1) * P)
                cos_t = pool_sim.tile([P, 512], BF16, tag="cos")
                euc_t = pool_sim.tile([P, 512], BF16, tag="euc")
                lrn_t = pool_sim.tile([P, 512], BF16, tag="lrn")
                # dot product
                psd = ps_mm.tile([P, 512], F32, tag="ps", name=f"psd_{bt}_{nch}")
                for kb in range(DB):
                    nc.tensor.matmul(psd, qpT[:, kb, bsl], kps[:, kb, :],
                                     start=(kb == 0), stop=(kb == DB - 1))
                # cosine: dot * (invq*et)[b] * invk[n]
                nc.vector.scalar_tensor_tensor(
                    out=cos_t, in0=psd, scalar=invq_et[:, bt:bt + 1],
                    in1=ivk_b[:, cols], op0=ALU.mult, op1=ALU.mult)
                # euclidean: 1/(1+sqrt(max(qsq+ksq-2dot, 0)))
                t = pool_work.tile([P, 512], F32, tag="eu1")
                nc.vector.scalar_tensor_tensor(
                    out=t, in0=psd, scalar=-2.0, in1=ksq_b[:, cols],
                    op0=ALU.mult, op1=ALU.add)
                nc.vector.tensor_scalar(out=t, in0=t,
                                        scalar1=qsq_col[:, bt:bt + 1],
                                        scalar2=0.0, op0=ALU.add, op1=ALU.max)
                s = pool_work.tile([P, 512], BF16, tag="eu2")
                nc.scalar.activation(s, t, AF.Sqrt)
                nc.vector.tensor_scalar_add(s, s, 1.0)
                nc.vector.reciprocal(euc_t, s)
                # learned: sum_h exp(score*scale - ln(denom*8))
                eh = pool_eh.tile([P, 512, H], BF16, tag="eh")
                for h in range(H):
                    pss = ps_mm.tile([P, 512], F32, tag="ps", name=f"pss_{bt}_{nch}_{h}")
                    nc.tensor.matmul(pss, qhT[:, h, bsl], khs[:, h, :],
                                     start=True, stop=True)
                    nc.scalar.activation(
                        eh[:, :, h], pss, AF.Exp, scale=SCALE,
                        bias=lnd[:, bt * H + h: bt * H + h + 1])
                nc.vector.tensor_reduce(out=lrn_t, in_=eh,
                                        axis=mybir.AxisListType.X, op=ALU.add)

                # ---- fusion MLP (see module docstring)
                pf_all = ps_out.tile([P, 4, P], F32, tag="pf",
                                     name=f"pf_{bt}_{nch}")
                for quar in range(4):    # 16 pairs per quarter
                    st6 = pool_stack.tile([6, 16, 512], BF16, tag="st6",
                                          name=f"st6_{bt}_{nch}_{quar}")
                    row0 = quar * 32
                    for ci, simt in enumerate((cos_t, euc_t, lrn_t)):
                        nc.sync.dma_start(
                            out=st6[ci:ci + 1, :, :],
                            in_=simt[row0:row0 + 32:2, :])
                        nc.sync.dma_start(
                            out=st6[ci + 3:ci + 4, :, :],
                            in_=simt[row0 + 1:row0 + 32:2, :])
                    h1s = []
                    for q in range(16):
                        ph = ps_h1.tile([P, 512], F32, tag="ph",
                                        name=f"ph_{bt}_{nch}_{quar}_{q}")
                        nc.tensor.matmul(ph, fw1blk, st6[:, q, :],
                                         start=True, stop=True)
                        h1 = pool_h1.tile([P, 512], BF16, tag=f"h1_{q}",
                                          name=f"h1_{bt}_{nch}_{quar}_{q}")
                        if q % 2 == 0:
                            nc.scalar.activation(h1, ph, AF.Relu, bias=fb1)
                        else:
                            nc.vector.tensor_scalar(
                                out=h1, in0=ph, scalar1=fb1, scalar2=0.0,
                                op0=ALU.add, op1=ALU.max)
                        h1s.append(h1)
                    for nblk in range(4):
                        bcols = slice(nblk * P, (nblk + 1) * P)
                        for i, h1 in enumerate(h1s):
                            pcol = quar * 32 + 2 * i
                            nc.tensor.matmul(pf_all[:, nblk, pcol:pcol + 2],
                                             h1[:, bcols], fw2,
                                             start=True, stop=True,
                                             skip_group_check=True)
                for nblk in range(4):
                    ot = pool_outs.tile([P, P], F32, tag="ot")
                    nc.scalar.activation(ot, pf_all[:, nblk, :], AF.Sigmoid,
                                         bias=fb2)
                    nc.sync.dma_start(
                        out=out_dram[nch * 512 + nblk * P:
                                     nch * 512 + (nblk + 1) * P, bsl],
                        in_=ot)
    ctx.close()


_CACHED = None


def _get_program():
    global _CACHED
    if _CACHED is None:
        _CACHED = build_program()
    return _CACHED


def _make_in_maps(inputs):
    in_maps = []
    for c in range(NCORES):
        m = {"query_features": np.ascontiguousarray(
                 inputs["query_features"], dtype=np.float32),
             "candidate_features": np.ascontiguousarray(
                 np.asarray(inputs["candidate_features"],
                            dtype=np.float32)[c * NL:(c + 1) * NL])}
        for k in WNAMES:
            v = np.asarray(inputs[k], dtype=np.float32)
            if k == "temperature":
                v = v.reshape([1])
            m[k] = np.ascontiguousarray(v)
        in_maps.append(m)
    return in_maps


def kernel(**inputs):
    nc = _get_program()
    in_maps = _make_in_maps(inputs)
    res = run_bass_kernel_spmd(nc, in_maps, core_ids=list(range(NCORES)))
    return np.concatenate(
        [np.ascontiguousarray(res.results[c]["out"].T) for c in range(NCORES)],
        axis=1)


def run_profiled(inputs):
    """Like kernel() but returns (output, exec_time_ns, trace_path)."""
    import os
    os.environ["BASS_PERFETTO_PROFILE_ALL_CORES"] = "1"
    nc = _get_program()
    in_maps = _make_in_maps(inputs)
    res = run_bass_kernel_spmd(nc, in_maps, core_ids=list(range(NCORES)),
                               trace=True, trace_cores=list(range(NCORES)))
    out = np.concatenate(
        [np.ascontiguousarray(res.results[c]["out"].T) for c in range(NCORES)],
        axis=1)
    tp = res.instructions_and_trace[1] if res.instructions_and_trace else None
    return out, res.exec_time_ns, tp


# revision 31
# speedup vs baseline: 1.3795x; 1.0010x over previous
"""Trainium2 Bass kernel for AdvancedSimilarityComputation (retrieval_knn).

Sharding: candidates (N=16384) split across 8 NeuronCores (2048 each).
Each core computes the full query projection (replicated) plus its candidate
shard's projections and the [B, N_local] similarity block; softmax over the
full N axis needs one tiny AllReduce of per-(query, head) exp-sums.

Layout: activations live TRANSPOSED — [d_model on partitions (8 blocks of
128), rows on free] — so every linear layer is matmul(out=Y^T, lhsT=W
(natural layout), rhs=X^T) with no per-layer transposes; only the raw
inputs are transposed once on the PE.  kp^T/kh^T are staged to DRAM and
streamed back per 512-candidate chunk (SBUF pressure).

The fusion MLP runs on the PE: L1 as one block-diagonal [6->128] matmul per
b-row pair (dual-row packing), relu-evicted to bf16 alternating ACT/DVE;
L2 uses the h1 tile as the matmul STATIONARY with block-diag [w2;w2] as the
moving operand so outputs land DENSE as [n-partitions, b-free].  The device
output is therefore [N_local, B]; the host transposes.
"""

import numpy as np
from contextlib import ExitStack

import concourse.bass as bass
import concourse.tile as tile
from concourse import bacc, mybir
from concourse.bass_utils import run_bass_kernel_spmd
from concourse.masks import make_identity

F32 = mybir.dt.float32
BF16 = mybir.dt.bfloat16
AF = mybir.ActivationFunctionType
ALU = mybir.AluOpType

B = 1024          # queries
D = 1024          # d_model
N = 16384         # candidates (global)
NCORES = 8
NL = N // NCORES  # candidates per core
H = 8
HD = D // H
P = 128
DB = D // P       # 8 d-model blocks
SCALE = 1.0 / float(np.sqrt(HD))
EPS = 1e-5
FH = 64           # fusion hidden

WNAMES = [
    "temperature",
    "q_w1", "q_b1", "q_g", "q_be", "q_w2", "q_b2",
    "k_w1", "k_b1", "k_g", "k_be", "k_w2", "k_b2",
    "wq", "bq", "wk", "bk",
    "f_w1", "f_b1", "f_w2", "f_b2",
]


def _bcast_ap(src_ap, nparts):
    """Partition-broadcast a [1, ...] AP to nparts partitions (step 0)."""
    return bass.AP(
        tensor=src_ap.tensor,
        offset=src_ap.offset,
        ap=[[0, nparts]] + [list(p) for p in src_ap.ap[1:]],
    )


def build_program():
    nc = bacc.Bacc("TRN2", target_bir_lowering=False, debug=False,
                   num_devices=NCORES)

    dram = {}
    dram["query_features"] = nc.dram_tensor(
        "query_features", [B, D], F32, kind="ExternalInput").ap()
    dram["candidate_features"] = nc.dram_tensor(
        "candidate_features", [NL, D], F32, kind="ExternalInput").ap()
    shapes = {
        "temperature": [1],
        "q_w1": [D, D], "q_b1": [D], "q_g": [D], "q_be": [D],
        "q_w2": [D, D], "q_b2": [D],
        "k_w1": [D, D], "k_b1": [D], "k_g": [D], "k_be": [D],
        "k_w2": [D, D], "k_b2": [D],
        "wq": [D, D], "bq": [D], "wk": [D, D], "bk": [D],
        "f_w1": [3, FH], "f_b1": [FH], "f_w2": [FH, 1], "f_b2": [1],
    }
    for k in WNAMES:
        dram[k] = nc.dram_tensor(k, shapes[k], F32, kind="ExternalInput").ap()
    # transposed on-device layout [NL, B]; the host transposes + concats
    out_dram = nc.dram_tensor("out", [NL, B], F32, kind="ExternalOutput").ap()

    with tile.TileContext(nc) as tc:
        with nc.allow_low_precision(reason="bf16 pipeline validated offline"):
            _build(nc, tc, dram, out_dram)
    nc.compile()
    return nc


def _build(nc, tc, dram, out_dram):
    ctx = ExitStack()
    pool_const = ctx.enter_context(tc.tile_pool(name="const", bufs=1))
    pool_big = ctx.enter_context(tc.tile_pool(name="big", bufs=1))
    ps_mm = ctx.enter_context(tc.tile_pool(name="ps_mm", bufs=2, space="PSUM"))
    dram_pool = ctx.enter_context(tc.tile_pool(name="dramp", bufs=1,
                                               space="DRAM"))

    def bcast_rows(row_ap, dst, tag):
        d = dram_pool.tile([1, row_ap.shape[-1]], row_ap.dtype, tag=tag,
                           name=f"bd_{tag}")
        nc.sync.dma_start(out=d, in_=row_ap)
        nc.gpsimd.dma_start(out=dst, in_=_bcast_ap(d[:], P))

    # ---- constants ----
    ident = pool_const.tile([P, P], F32)
    make_identity(nc, ident)
    ones_bf = pool_const.tile([P, 1], BF16)
    nc.vector.memset(ones_bf, 1.0)
    eps_t = pool_const.tile([1, 1], F32)
    nc.vector.memset(eps_t, EPS)

    def load_colvec(vap):
        t = pool_const.tile([P, vap.shape[0] // P], F32,
                            name=f"cv_{vap.tensor.name}")
        nc.sync.dma_start(out=t, in_=vap.rearrange("(blk p) -> p blk", p=P))
        return t

    b1q = load_colvec(dram["q_b1"]); gq = load_colvec(dram["q_g"])
    beq = load_colvec(dram["q_be"]); b2q = load_colvec(dram["q_b2"])
    b1k = load_colvec(dram["k_b1"]); gk = load_colvec(dram["k_g"])
    bek = load_colvec(dram["k_be"]); b2k = load_colvec(dram["k_b2"])
    bqc = load_colvec(dram["bq"]); bkc = load_colvec(dram["bk"])

    # fusion weights: block-diag dual-row L1 [6,128] and L2 moving [w2;w2]
    fw1bf = pool_const.tile([6, P], F32)
    nc.vector.memset(fw1bf, 0.0)
    nc.sync.dma_start(out=fw1bf[0:3, 0:FH], in_=dram["f_w1"])
    nc.sync.dma_start(out=fw1bf[3:6, FH:P], in_=dram["f_w1"])
    fw1blk = pool_const.tile([6, P], BF16)
    nc.vector.tensor_copy(out=fw1blk, in_=fw1bf)
    fw2f = pool_const.tile([P, 2], F32)
    nc.vector.memset(fw2f, 0.0)
    nc.sync.dma_start(out=fw2f[0:FH, 0:1], in_=dram["f_w2"])
    nc.sync.dma_start(out=fw2f[FH:P, 1:2], in_=dram["f_w2"])
    fw2 = pool_const.tile([P, 2], BF16)
    nc.vector.tensor_copy(out=fw2, in_=fw2f)
    fb1 = pool_const.tile([P, 1], F32)
    nc.sync.dma_start(out=fb1[0:FH, :],
                      in_=dram["f_b1"].rearrange("(p o) -> p o", o=1))
    nc.sync.dma_start(out=fb1[FH:P, :],
                      in_=dram["f_b1"].rearrange("(p o) -> p o", o=1))
    fb2 = pool_const.tile([P, 1], F32)
    nc.gpsimd.dma_start(out=fb2, in_=_bcast_ap(
        dram["f_b2"].rearrange("(o p) -> o p", o=1), P))

    tmp_t = pool_const.tile([1, 1], F32)
    nc.sync.dma_start(out=tmp_t,
                      in_=dram["temperature"].rearrange("(o p) -> o p", o=1))
    et_row = pool_const.tile([1, 1], F32)
    nc.scalar.activation(et_row, tmp_t, AF.Exp)
    et_b = pool_const.tile([P, 1], F32)
    bcast_rows(et_row, et_b, "et")

    # ---- persistent SBUF (q side) + DRAM staging (k side) ----
    qpT = pool_big.tile([P, DB, B], BF16)
    qhT = pool_big.tile([P, DB, B], BF16)
    kp_dram = dram_pool.tile([P, DB, NL], BF16)
    kh_dram = dram_pool.tile([P, DB, NL], BF16)

    # norm-derived constants (filled in phase A)
    invq_et = pool_const.tile([P, B // P], F32)
    qsq_col = pool_const.tile([P, B // P], F32)
    ksq_b = pool_const.tile([P, NL], BF16)
    ivk_b = pool_const.tile([P, NL], BF16)

    # =====================================================================
    # PHASE A: projections (phase-scoped pools)
    # =====================================================================
    with ExitStack() as actx:
        pool_w = actx.enter_context(tc.tile_pool(name="wpool", bufs=2))
        pool_stage = actx.enter_context(tc.tile_pool(name="stage", bufs=2))
        pool_x = actx.enter_context(tc.tile_pool(name="xpool", bufs=1))
        pool_t1 = actx.enter_context(tc.tile_pool(name="t1", bufs=1))
        pool_work = actx.enter_context(tc.tile_pool(name="workA", bufs=2))
        pool_small = actx.enter_context(tc.tile_pool(name="smallA", bufs=1))
        pool_bc = actx.enter_context(tc.tile_pool(name="bcA", bufs=2))
        pool_kch = actx.enter_context(tc.tile_pool(name="kch", bufs=2))
        ps_tr = actx.enter_context(tc.tile_pool(name="ps_tr", bufs=2,
                                                space="PSUM"))
        ps_stat = actx.enter_context(tc.tile_pool(name="ps_stat", bufs=1,
                                                  space="PSUM"))

        qsq_row = pool_small.tile([1, B], F32, tag="qsq_row")
        ksq_row = pool_small.tile([1, NL], F32, tag="ksq_row")

        def load_weight(wap, slot):
            wt = pool_w.tile([P, DB, D], BF16, tag="wcur",
                             name=f"w_{wap.tensor.name}_{slot}")
            for kb in range(DB):
                st = pool_stage.tile([P, D], F32, tag="stg")
                nc.sync.dma_start(out=st, in_=wap[kb * P:(kb + 1) * P, :])
                nc.any.tensor_copy(out=wt[:, kb, :], in_=st)
            return wt

        def transpose_input(xap, r0, R, name):
            xT = pool_x.tile([P, DB, R], BF16, tag="xT", name=name)
            for rb in range(R // P):
                st = pool_stage.tile([P, D], F32, tag="stg")
                nc.sync.dma_start(out=st,
                                  in_=xap[r0 + rb * P:r0 + (rb + 1) * P, :])
                for db in range(DB):
                    pt = ps_tr.tile([P, P], F32, tag="pt")
                    nc.tensor.transpose(pt, st[:, db * P:(db + 1) * P], ident)
                    nc.any.tensor_copy(out=xT[:, db, rb * P:(rb + 1) * P],
                                       in_=pt)
            return xT

        def projection(xT, R, cb, w1, w2, b1t, gt, bet, b2t, outT, oc0,
                       sq_row):
            """Project R rows; write outT[:, :, oc0:oc0+R], sq_row[cb:]."""
            t2T = pool_t1.tile([P, DB, R], BF16, tag="t2T",
                               name=f"t2T_{cb}_{outT.name}")
            for rb in range(R // 512):
                cols = slice(rb * 512, (rb + 1) * 512)
                ps_mu = ps_stat.tile([1, 512], F32, tag="ps_mu")
                ps_sq = ps_stat.tile([1, 512], F32, tag="ps_sq")
                t1f = pool_t1.tile([P, DB, 512], BF16, tag="t1f", bufs=2)
                for mb in range(DB):
                    ps = ps_mm.tile([P, 512], F32, tag="ps")
                    for kb in range(DB):
                        nc.tensor.matmul(ps, w1[:, kb, mb * P:(mb + 1) * P],
                                         xT[:, kb, cols],
                                         start=(kb == 0), stop=(kb == DB - 1))
                    nc.scalar.activation(t1f[:, mb, :], ps, AF.Identity,
                                         bias=b1t[:, mb:mb + 1])
                    sq = pool_work.tile([P, 512], BF16, tag="sq")
                    nc.vector.tensor_mul(sq, t1f[:, mb, :], t1f[:, mb, :])
                    nc.tensor.matmul(ps_mu, ones_bf, t1f[:, mb, :],
                                     start=(mb == 0), stop=(mb == DB - 1),
                                     skip_group_check=True)
                    nc.tensor.matmul(ps_sq, ones_bf, sq,
                                     start=(mb == 0), stop=(mb == DB - 1),
                                     skip_group_check=True)
                mu = pool_small.tile([1, 512], F32, tag="mu")
                nc.scalar.mul(mu, ps_mu, 1.0 / D)
                msq = pool_small.tile([1, 512], F32, tag="msq")
                nc.scalar.mul(msq, ps_sq, 1.0 / D)
                var = pool_small.tile([1, 512], F32, tag="var")
                nc.vector.tensor_mul(var, mu, mu)
                nc.vector.tensor_tensor(out=var, in0=msq, in1=var,
                                        op=ALU.subtract)
                sd = pool_small.tile([1, 512], F32, tag="sd")
                nc.scalar.activation(sd, var, AF.Sqrt, bias=eps_t)
                rstd = pool_small.tile([1, 512], F32, tag="rstd")
                nc.vector.reciprocal(rstd, sd)
                mu_bf = pool_small.tile([1, 512], BF16, tag="mu_bf")
                nc.vector.tensor_copy(out=mu_bf, in_=mu)
                rstd_bf = pool_small.tile([1, 512], BF16, tag="rstd_bf")
                nc.vector.tensor_copy(out=rstd_bf, in_=rstd)
                mu_b = pool_bc.tile([P, 512], BF16, tag="mu_b")
                bcast_rows(mu_bf, mu_b, "mu_d")
                rstd_b = pool_bc.tile([P, 512], BF16, tag="rstd_b")
                bcast_rows(rstd_bf, rstd_b, "rstd_d")
                for mb in range(DB):
                    u = pool_work.tile([P, 512], BF16, tag="u")
                    nc.vector.tensor_tensor(out=u, in0=t1f[:, mb, :],
                                            in1=mu_b, op=ALU.subtract)
                    nc.vector.tensor_mul(u, u, rstd_b)
                    nc.scalar.activation(t2T[:, mb, cols], u, AF.Gelu,
                                         bias=bet[:, mb:mb + 1],
                                         scale=gt[:, mb:mb + 1])
            for rb in range(R // 512):
                cols = slice(rb * 512, (rb + 1) * 512)
                wcols = slice(oc0 + rb * 512, oc0 + (rb + 1) * 512)
                scols = slice(cb + rb * 512, cb + (rb + 1) * 512)
                ps_ss = ps_stat.tile([1, 512], F32, tag="ps_mu", name=f"ps_ss_{cb}_{rb}_{outT.name}")
                for mb in range(DB):
                    ps = ps_mm.tile([P, 512], F32, tag="ps")
                    for kb in range(DB):
                        nc.tensor.matmul(ps, w2[:, kb, mb * P:(mb + 1) * P],
                                         t2T[:, kb, cols],
                                         start=(kb == 0), stop=(kb == DB - 1))
                    nc.scalar.activation(outT[:, mb, wcols], ps, AF.Identity,
                                         bias=b2t[:, mb:mb + 1])
                    sq = pool_work.tile([P, 512], BF16, tag="sq")
                    nc.vector.tensor_mul(sq, outT[:, mb, wcols],
                                         outT[:, mb, wcols])
                    nc.tensor.matmul(ps_ss, ones_bf, sq,
                                     start=(mb == 0), stop=(mb == DB - 1),
                                     skip_group_check=True)
                nc.vector.tensor_copy(out=sq_row[0:1, scols], in_=ps_ss)

        def head_proj(xT, R, w, bt_col, outT):
            for rb in range(R // 512):
                cols = slice(rb * 512, (rb + 1) * 512)
                for mb in range(DB):
                    ps = ps_mm.tile([P, 512], F32, tag="ps")
                    for kb in range(DB):
                        nc.tensor.matmul(ps, w[:, kb, mb * P:(mb + 1) * P],
                                         xT[:, kb, cols],
                                         start=(kb == 0), stop=(kb == DB - 1))
                    nc.scalar.activation(outT[:, mb, cols], ps, AF.Identity,
                                         bias=bt_col[:, mb:mb + 1])

        # -- query side (stays in SBUF), 512-row chunks
        w1q = load_weight(dram["q_w1"], 0)
        w2q = load_weight(dram["q_w2"], 1)
        for rch in range(B // 512):
            xqT = transpose_input(dram["query_features"], rch * 512, 512,
                                  f"xqT{rch}")
            projection(xqT, 512, rch * 512, w1q, w2q, b1q, gq, beq, b2q,
                       qpT, rch * 512, qsq_row)

        # -- candidate side, 512-row chunks staged to DRAM
        w1k = load_weight(dram["k_w1"], 2)
        w2k = load_weight(dram["k_w2"], 3)
        for rch in range(NL // 512):
            xcT = transpose_input(dram["candidate_features"], rch * 512, 512,
                                  f"xcT{rch}")
            kch = pool_kch.tile([P, DB, 512], BF16, tag="kch",
                                name=f"kp_ch{rch}")
            projection(xcT, 512, rch * 512, w1k, w2k, b1k, gk, bek, b2k,
                       kch, 0, ksq_row)
            nc.sync.dma_start(out=kp_dram[:, :, rch * 512:(rch + 1) * 512],
                              in_=kch)

        wqw = load_weight(dram["wq"], 4)
        head_proj(qpT, B, wqw, bqc, qhT)
        wkw = load_weight(dram["wk"], 5)
        for rch in range(NL // 512):
            kch = pool_kch.tile([P, DB, 512], BF16, tag="kch",
                                name=f"kp_rd{rch}")
            nc.sync.dma_start(out=kch,
                              in_=kp_dram[:, :, rch * 512:(rch + 1) * 512])
            khch = pool_kch.tile([P, DB, 512], BF16, tag="khch",
                                 name=f"kh_ch{rch}")
            head_proj(kch, 512, wkw, bkc, khch)
            nc.sync.dma_start(out=kh_dram[:, :, rch * 512:(rch + 1) * 512],
                              in_=khch)

        # -- norms / scale vectors (rows derived in place)
        for bt in range(B // P):
            pt2 = ps_tr.tile([P, 1], F32, tag="pt", name=f"pt2_{bt}")
            nc.tensor.transpose(pt2, qsq_row[0:1, bt * P:(bt + 1) * P],
                                ident[0:1, 0:1])
            nc.any.tensor_copy(out=qsq_col[:, bt:bt + 1], in_=pt2)
        ksq_bf = pool_small.tile([1, NL], BF16, tag="ksq_bf")
        nc.vector.tensor_copy(out=ksq_bf, in_=ksq_row)
        bcast_rows(ksq_bf, ksq_b, "ksq_d")
        # overwrite the sq rows with 1/sqrt
        nc.scalar.activation(qsq_row, qsq_row, AF.Sqrt)
        nc.vector.reciprocal(qsq_row, qsq_row)
        nc.scalar.activation(ksq_row, ksq_row, AF.Sqrt)
        nc.vector.reciprocal(ksq_row, ksq_row)
        for bt in range(B // P):
            pt1 = ps_tr.tile([P, 1], F32, tag="pt", name=f"pt1_{bt}")
            nc.tensor.transpose(pt1, qsq_row[0:1, bt * P:(bt + 1) * P],
                                ident[0:1, 0:1])
            nc.scalar.mul(invq_et[:, bt:bt + 1], pt1, et_b[:, 0:1])
        ivk_bf = pool_small.tile([1, NL], BF16, tag="ksq_bf",
                                 name="ivk_bf")
        nc.vector.tensor_copy(out=ivk_bf, in_=ksq_row)
        bcast_rows(ivk_bf, ivk_b, "ivk_d")

    # =====================================================================
    # PHASE B: similarity passes + fusion MLP (nch-outer, k streamed)
    # =====================================================================
    n_bt = B // P
    n_nch = NL // 512
    with ExitStack() as bctx:
        pool_ks = bctx.enter_context(tc.tile_pool(name="ks", bufs=2))
        pool_work = bctx.enter_context(tc.tile_pool(name="workB", bufs=2))
        pool_small = bctx.enter_context(tc.tile_pool(name="smallB", bufs=1))
        pool_sim = bctx.enter_context(tc.tile_pool(name="sim", bufs=3))
        pool_stack = bctx.enter_context(tc.tile_pool(name="stack", bufs=2))
        pool_h1 = bctx.enter_context(tc.tile_pool(name="h1", bufs=2))
        pool_eh = bctx.enter_context(tc.tile_pool(name="eh", bufs=2))
        pool_outs = bctx.enter_context(tc.tile_pool(name="outs", bufs=2))
        ps_h1 = bctx.enter_context(tc.tile_pool(name="ps_h1", bufs=2,
                                                space="PSUM"))
        ps_out = bctx.enter_context(tc.tile_pool(name="ps_out", bufs=2,
                                                 space="PSUM"))

        # ---- pass 1: softmax denominators (nch-outer, khT streamed)
        rs_all = pool_const.tile([P, n_bt * H, n_nch], F32)
        for nch in range(n_nch):
            cols = slice(nch * 512, (nch + 1) * 512)
            khs = pool_ks.tile([P, DB, 512], BF16, tag="khs",
                               name=f"khs_p1_{nch}")
            nc.sync.dma_start(out=khs, in_=kh_dram[:, :, cols])
            for bt in range(n_bt):
                bsl = slice(bt * P, (bt + 1) * P)
                for h in range(H):
                    ps = ps_mm.tile([P, 512], F32, tag="ps")
                    nc.tensor.matmul(ps, qhT[:, h, bsl], khs[:, h, :],
                                     start=True, stop=True)
                    junk = pool_work.tile([P, 512], BF16, tag="junk")
                    nc.scalar.activation(
                        junk, ps, AF.Exp, scale=SCALE,
                        accum_out=rs_all[:, bt * H + h, nch:nch + 1])
        rssum = pool_const.tile([P, n_bt * H], F32)
        nc.vector.tensor_reduce(out=rssum, in_=rs_all,
                                axis=mybir.AxisListType.X, op=ALU.add)

        # ---- collective: AllReduce the denominators
        cc_in = dram_pool.tile([P, n_bt * H], F32)
        cc_out = dram_pool.tile([P, n_bt * H], F32)
        nc.sync.dma_start(out=cc_in, in_=rssum)
        nc.gpsimd.collective_compute(
            "AllReduce", ALU.add,
            replica_groups=[list(range(NCORES))],
            ins=[cc_in.opt()],
            outs=[cc_out.opt()],
        )
        denom = pool_const.tile([P, n_bt * H], F32)
        nc.sync.dma_start(out=denom, in_=cc_out)
        # bias for pass2 exp: -(ln denom) - ln 8 (folds the mean over heads)
        lnd = pool_const.tile([P, n_bt * H], F32)
        nc.scalar.activation(lnd, denom, AF.Ln)
        nc.vector.tensor_scalar(out=lnd, in0=lnd, scalar1=-1.0,
                                scalar2=-float(np.log(H)), op0=ALU.mult,
                                op1=ALU.add)

        # ---- pass 2 + fusion MLP, software-pipelined: sims(i) || MLP(i-1)
        def emit_kload(nch):
            cols = slice(nch * 512, (nch + 1) * 512)
            kps = pool_ks.tile([P, DB, 512], BF16, tag="kps",
                               name=f"kps_{nch}")
            nc.sync.dma_start(out=kps, in_=kp_dram[:, :, cols])
            khs = pool_ks.tile([P, DB, 512], BF16, tag="khs",
                               name=f"khs_p2_{nch}")
            nc.sync.dma_start(out=khs, in_=kh_dram[:, :, cols])
            return kps, khs

        def emit_sims(nch, bt, kps, khs):
            cols = slice(nch * 512, (nch + 1) * 512)
            bsl = slice(bt * P, (bt + 1) * P)
            cos_t = pool_sim.tile([P, 512], BF16, tag="cos",
                                  name=f"cos_{bt}_{nch}")
            euc_t = pool_sim.tile([P, 512], BF16, tag="euc",
                                  name=f"euc_{bt}_{nch}")
            lrn_t = pool_sim.tile([P, 512], BF16, tag="lrn",
                                  name=f"lrn_{bt}_{nch}")
            # dot product
            psd = ps_mm.tile([P, 512], F32, tag="ps", name=f"psd_{bt}_{nch}")
            for kb in range(DB):
                nc.tensor.matmul(psd, qpT[:, kb, bsl], kps[:, kb, :],
                                 start=(kb == 0), stop=(kb == DB - 1))
            # cosine: dot * (invq*et)[b] * invk[n]
            nc.vector.scalar_tensor_tensor(
                out=cos_t, in0=psd, scalar=invq_et[:, bt:bt + 1],
                in1=ivk_b[:, cols], op0=ALU.mult, op1=ALU.mult)
            # learned: sum_h exp(score*scale - ln(denom*8))
            eh = pool_eh.tile([P, 512, H], BF16, tag="eh",
                              name=f"eh_{bt}_{nch}")
            for h in range(H):
                pss = ps_mm.tile([P, 512], F32, tag="ps",
                                 name=f"pss_{bt}_{nch}_{h}")
                nc.tensor.matmul(pss, qhT[:, h, bsl], khs[:, h, :],
                                 start=True, stop=True)
                nc.scalar.activation(
                    eh[:, :, h], pss, AF.Exp, scale=SCALE,
                    bias=lnd[:, bt * H + h: bt * H + h + 1])
            nc.vector.tensor_reduce(out=lrn_t, in_=eh,
                                    axis=mybir.AxisListType.X, op=ALU.add)
            # euclidean: 1/(1+sqrt(max(qsq+ksq-2dot, 0)))
            t = pool_work.tile([P, 512], F32, tag="eu1",
                               name=f"eu1_{bt}_{nch}")
            nc.vector.scalar_tensor_tensor(
                out=t, in0=psd, scalar=-2.0, in1=ksq_b[:, cols],
                op0=ALU.mult, op1=ALU.add)
            nc.vector.tensor_scalar(out=t, in0=t,
                                    scalar1=qsq_col[:, bt:bt + 1],
                                    scalar2=0.0, op0=ALU.add, op1=ALU.max)
            s = pool_work.tile([P, 512], BF16, tag="eu2",
                               name=f"eu2_{bt}_{nch}")
            nc.scalar.activation(s, t, AF.Sqrt)
            nc.vector.tensor_scalar_add(s, s, 1.0)
            nc.vector.reciprocal(euc_t, s)
            return cos_t, euc_t, lrn_t

        def emit_mlp(nch, bt, sims):
            cos_t, euc_t, lrn_t = sims
            bsl = slice(bt * P, (bt + 1) * P)
            pf_all = ps_out.tile([P, 4, P], F32, tag="pf",
                                 name=f"pf_{bt}_{nch}")
            for quar in range(4):    # 16 pairs per quarter
                st6 = pool_stack.tile([6, 16, 512], BF16, tag="st6",
                                      name=f"st6_{bt}_{nch}_{quar}")
                row0 = quar * 32
                for ci, simt in enumerate((cos_t, euc_t, lrn_t)):
                    nc.sync.dma_start(
                        out=st6[ci:ci + 1, :, :],
                        in_=simt[row0:row0 + 32:2, :])
                    nc.sync.dma_start(
                        out=st6[ci + 3:ci + 4, :, :],
                        in_=simt[row0 + 1:row0 + 32:2, :])
                h1s = []
                for q2 in range(8):      # two pairs per psum/evict batch
                    ph = ps_h1.tile([P, 2, 512], F32, tag="ph",
                                    name=f"ph_{bt}_{nch}_{quar}_{q2}")
                    nc.tensor.matmul(ph[:, 0, :], fw1blk,
                                     st6[:, 2 * q2, :],
                                     start=True, stop=True,
                                     skip_group_check=True)
                    nc.tensor.matmul(ph[:, 1, :], fw1blk,
                                     st6[:, 2 * q2 + 1, :],
                                     start=True, stop=True,
                                     skip_group_check=True)
                    h1 = pool_h1.tile([P, 2, 512], BF16, tag=f"h1_{q2}",
                                      name=f"h1_{bt}_{nch}_{quar}_{q2}")
                    if (quar * 8 + q2) % 16 < 9:
                        nc.scalar.activation(h1, ph, AF.Relu, bias=fb1)
                    else:
                        nc.vector.tensor_scalar(
                            out=h1, in0=ph, scalar1=fb1, scalar2=0.0,
                            op0=ALU.add, op1=ALU.max)
                    h1s.append(h1)
                for nblk in range(4):
                    bcols = slice(nblk * P, (nblk + 1) * P)
                    for q2, h1 in enumerate(h1s):
                        for par in range(2):
                            pcol = quar * 32 + 4 * q2 + 2 * par
                            nc.tensor.matmul(
                                pf_all[:, nblk, pcol:pcol + 2],
                                h1[:, par, bcols], fw2,
                                start=True, stop=True,
                                skip_group_check=True)
            for nblk in range(4):
                ot = pool_outs.tile([P, P], F32, tag="ot",
                                    name=f"ot_{bt}_{nch}_{nblk}")
                nc.scalar.activation(ot, pf_all[:, nblk, :], AF.Sigmoid,
                                     bias=fb2)
                nc.sync.dma_start(
                    out=out_dram[nch * 512 + nblk * P:
                                 nch * 512 + (nblk + 1) * P, bsl],
                    in_=ot)

        iters = [(nch, bt) for nch in range(n_nch) for bt in range(n_bt)]
        prev = None
        kmap = {}
        for idx, (nch, bt) in enumerate(iters):
            if bt == 0:
                kmap[nch] = emit_kload(nch)
            sims = emit_sims(nch, bt, *kmap[nch])
            if prev is not None:
                emit_mlp(prev[0], prev[1], prev[2])
            prev = (nch, bt, sims)
        emit_mlp(prev[0], prev[1], prev[2])
    ctx.close()


_CACHED = None


def _get_program():
    global _CACHED
    if _CACHED is None:
        _CACHED = build_program()
    return _CACHED


def _make_in_maps(inputs):
    in_maps = []
    for c in range(NCORES):
        m = {"query_features": np.ascontiguousarray(
                 inputs["query_features"], dtype=np.float32),
             "candidate_features": np.ascontiguousarray(
                 np.asarray(inputs["candidate_features"],
                            dtype=np.float32)[c * NL:(c + 1) * NL])}
        for k in WNAMES:
            v = np.asarray(inputs[k], dtype=np.float32)
            if k == "temperature":
                v = v.reshape([1])
            m[k] = np.ascontiguousarray(v)
        in_maps.append(m)
    return in_maps


def kernel(**inputs):
    nc = _get_program()
    in_maps = _make_in_maps(inputs)
    res = run_bass_kernel_spmd(nc, in_maps, core_ids=list(range(NCORES)))
    return np.concatenate(
        [np.ascontiguousarray(res.results[c]["out"].T) for c in range(NCORES)],
        axis=1)


def run_profiled(inputs):
    """Like kernel() but returns (output, exec_time_ns, trace_path)."""
    import os
    os.environ["BASS_PERFETTO_PROFILE_ALL_CORES"] = "1"
    nc = _get_program()
    in_maps = _make_in_maps(inputs)
    res = run_bass_kernel_spmd(nc, in_maps, core_ids=list(range(NCORES)),
                               trace=True, trace_cores=list(range(NCORES)))
    out = np.concatenate(
        [np.ascontiguousarray(res.results[c]["out"].T) for c in range(NCORES)],
        axis=1)
    tp = res.instructions_and_trace[1] if res.instructions_and_trace else None
    return out, res.exec_time_ns, tp
